# revision 53
# baseline (speedup 1.0000x reference)
"""Multi-head cross-modal attention + residual + LayerNorm on 8 TRN2 cores.

Reference computation (per batch b):
  Q = query @ Wq.T + bq ; K = key @ Wk.T + bk ; V = value @ Wv.T + bv
  attn = softmax(Q K^T / sqrt(D)) per head
  out  = (attn V) @ Wo.T + bo
  y    = LayerNorm(out + query) * gamma + beta

Sharding: 2-D over (batch=4) x (query-half=2). Core c owns batch c//2 and
queries [512*(c%2), 512*(c%2)+512); it computes ALL heads for its query
rows (K/V projections are duplicated across the pair of cores sharing a
batch), so there is NO collective and no partial-sum exchange: each core
emits its own 512 rows of the final LayerNorm output.

Precision/PE strategy: every projection and the attn@V / out-proj matmuls
run as fp8 (e4m3 data, e5m2 softmax weights) with MatmulPerfMode.DoubleRow,
pairing two 128-row contraction tiles per instruction (the pair lives in a
free-dim slot, so ordinary PSUM-copy layouts produce it). Weight matrices
are pre-scaled x32 on the host so their entries are ~N(0,1) in fp8; the
1/32 (projections) and 1/256 (out-proj, which also carries a x8 ctx scale)
rescales fold into existing bias/residual-add ops. Scores stay bf16
(contraction=64 cannot pair tiles), computed transposed per key-block with
a ones-column in V producing the softmax denominator as PSUM row 64.

Softmax: no max-subtraction (|score| < 7, fp32/e5m2 exp cannot overflow).
exp is split across three engines to break the ScalarE bottleneck: ACT
runs hardware Exp -> e5m2; DVE and GPSIMD run a Schraudolph-style bit
trick (y = round(a*score + b) written as int8, bit-identical to e5m2)
whose systematic component cancels in the softmax ratio. Per-head
normalization: DVE reciprocal of the denominator row, replicated down 64
partitions by a K=1 matmul with an 8.0-valued row (folding the x8 fp8
scale), then one scalar_tensor_tensor into the fp8 ctx tile.
"""

import sys

if "/opt/trn_rl_repo" not in sys.path:
    sys.path.insert(0, "/opt/trn_rl_repo")

import ml_dtypes
import numpy as np

import concourse.bass as bass  # noqa: F401  (registers types)
import concourse.mybir as mybir
import concourse.tile as tile
from concourse import bacc
from concourse.bass_utils import run_bass_kernel_spmd

F32 = mybir.dt.float32
F32R = mybir.dt.float32r
BF16 = mybir.dt.bfloat16
F8E4 = mybir.dt.float8e4
F8E5 = mybir.dt.float8e5
I8 = mybir.dt.int8
AF = mybir.ActivationFunctionType
OP = mybir.AluOpType
DR = mybir.MatmulPerfMode.DoubleRow

B, SQ, SK, E, H, D = 4, 1024, 2048, 1024, 16, 64
N_CORES = 8
QR = 512           # queries per core
EPS = 1e-5
WSCALE = 32.0      # host-side weight pre-scale for fp8
CTXSCALE = 8.0     # fp8 scale applied to normalized ctx

# Schraudolph exp -> e5m2 bits: bits = trunc(EXPA * raw_score + EXPB)
# raw_score = QK dot (pre 1/sqrt(64)); fitted offset, see build notes.
EXPA = 0.125 * 4.0 / float(np.log(2.0))
EXPB = 58.5

# exp engine split per (head, jt-pair): A=ACT hw exp, D=DVE bit-trick.
# (GPSIMD cannot read PSUM and DMA cannot stage from PSUM, so the softmax
# exp is strictly an ACT/DVE affair.)
ENG_EARLY = "ADADADDA"  # heads 0-7: A4 D4 (ACT also runs K/V copies)
ENG_LATE = "AADAADDA"   # heads 8-15: A5 D3 (end on A: DVE free for recip)

# module-level knobs used by test.py (harness ignores them)
TRACE = False
LAST_RESULTS = None

_NC_CACHE = None


def _build_nc():
    nc = bacc.Bacc(None, target_bir_lowering=False)

    q8 = nc.dram_tensor("q8", [128, 8 * QR], F8E4, kind="ExternalInput")
    k8 = nc.dram_tensor("k8", [128, 8 * SK], F8E4, kind="ExternalInput")
    v8 = nc.dram_tensor("v8", [128, 16 * 8 * 128], F8E4, kind="ExternalInput")
    wq8 = nc.dram_tensor("wq8", [128, 8 * E], F8E4, kind="ExternalInput")
    wk8 = nc.dram_tensor("wk8", [128, 8 * E], F8E4, kind="ExternalInput")
    wv8 = nc.dram_tensor("wv8", [128, 8 * E], F8E4, kind="ExternalInput")
    wo8 = nc.dram_tensor("wo8", [128, 8 * E], F8E4, kind="ExternalInput")
    bq8 = nc.dram_tensor("bq8", [128, 8], F32, kind="ExternalInput")
    bk8 = nc.dram_tensor("bk8", [128, 8], F32, kind="ExternalInput")
    resid = nc.dram_tensor("resid", [QR, E], BF16, kind="ExternalInput")
    vec3 = nc.dram_tensor("vec3", [2, E], BF16, kind="ExternalInput")
    ident = nc.dram_tensor("ident", [128, 128], BF16, kind="ExternalInput")
    out = nc.dram_tensor("out", [QR, E], BF16, kind="ExternalOutput")

    from contextlib import ExitStack

    with ExitStack() as ctx:
        tc = ctx.enter_context(tile.TileContext(nc))
        constp = ctx.enter_context(tc.tile_pool(name="consts", bufs=1))
        inp = ctx.enter_context(tc.tile_pool(name="inp", bufs=1))
        qtp = ctx.enter_context(tc.tile_pool(name="qtp", bufs=8))
        ktp = ctx.enter_context(tc.tile_pool(name="ktp", bufs=8))
        vsb = ctx.enter_context(tc.tile_pool(name="vsb", bufs=8))
        expp = ctx.enter_context(tc.tile_pool(name="expp", bufs=4))
        ctxp = ctx.enter_context(tc.tile_pool(name="ctxp", bufs=1))
        recp = ctx.enter_context(tc.tile_pool(name="recp", bufs=2))
        lnp = ctx.enter_context(tc.tile_pool(name="lnp", bufs=2))
        # one 6-bank pool serves scores pairs, projection groups and the
        # out-proj epilogue; pc accumulators get the other 2 banks.
        psc = ctx.enter_context(tc.tile_pool(name="psc", bufs=3, space="PSUM"))
        pcp = ctx.enter_context(tc.tile_pool(name="pcp", bufs=2, space="PSUM"))
        if True:
            # ---------------- input DMAs (need-order, chunked) ----------------
            # tiny bias vectors first (they gate the Q/K projection copies),
            # then k8/v8 stream in 4 chunks interleaved with the weights so
            # head 0 can start as soon as its first key-block band lands.
            bq_sb = constp.tile([128, 8], F32)
            bk_sb = constp.tile([128, 8], F32)
            nc.sync.dma_start(out=bq_sb, in_=bq8[:, :])
            nc.sync.dma_start(out=bk_sb, in_=bk8[:, :])
            wq_t = inp.tile([128, 8 * E], F8E4, tag="wq")
            nc.sync.dma_start(out=wq_t, in_=wq8[:, :])
            q_t = inp.tile([128, 8 * QR], F8E4, tag="q")
            nc.sync.dma_start(out=q_t, in_=q8[:, :])
            wk_t = inp.tile([128, 8 * E], F8E4, tag="wk")
            nc.sync.dma_start(out=wk_t, in_=wk8[:, :])
            k_t = inp.tile([128, 8 * SK], F8E4, tag="k")
            kvd = k_t.rearrange("p (s c) -> p s c", s=8)
            k8d = k8[:, :].rearrange("p (s c) -> p s c", s=8)
            wv_t = inp.tile([128, 8 * E], F8E4, tag="wv")
            v_t = inp.tile([128, 16 * 8 * 128], F8E4, tag="v")
            vvd = v_t.rearrange("p (jb r) -> p jb r", jb=16)
            v8d = v8[:, :].rearrange("p (jb r) -> p jb r", jb=16)
            nc.sync.dma_start(
                out=kvd[:, :, 0:512], in_=k8d[:, :, 0:512]
            )
            nc.sync.dma_start(out=wv_t, in_=wv8[:, :])
            for jc in range(4):
                if jc > 0:
                    nc.sync.dma_start(
                        out=kvd[:, :, jc * 512 : (jc + 1) * 512],
                        in_=k8d[:, :, jc * 512 : (jc + 1) * 512],
                    )
                nc.sync.dma_start(
                    out=vvd[:, 4 * jc : 4 * jc + 4, :],
                    in_=v8d[:, 4 * jc : 4 * jc + 4, :],
                )

            # slot views: (partition, slot, col)
            qv = q_t.rearrange("p (s c) -> p s c", s=8)
            kv = k_t.rearrange("p (s c) -> p s c", s=8)
            vv = v_t.rearrange("p (jb s c) -> p jb s c", jb=16, s=8)
            wqv = wq_t.rearrange("p (s c) -> p s c", s=8)
            wkv = wk_t.rearrange("p (s c) -> p s c", s=8)
            wvv = wv_t.rearrange("p (s c) -> p s c", s=8)

            def proj_psum(name, fromfill):
                if fromfill:
                    t2 = psc.tile([128, 2, 512], F32, tag="sc", name=name)
                    return t2[:, 0, :]
                return pcp.tile([128, 512], F32, tag="pc", name=name)[:]

            # ---------------- Q projection: QT[d, i] bf16 ----------------
            QTt = [
                qtp.tile([128, QR], BF16, tag="qt", name=f"QT_{t}")
                for t in range(8)
            ]
            for t in range(8):
                pq = proj_psum(f"pq_{t}", False)
                for p in range(4):
                    nc.tensor.matmul(
                        pq,
                        wqv[:, 2 * p : 2 * p + 2, t * 128 : (t + 1) * 128],
                        qv[:, 2 * p : 2 * p + 2, :],
                        start=(p == 0),
                        stop=(p == 3),
                        perf_mode=DR,
                    )
                nc.scalar.activation(
                    out=QTt[t][:],
                    in_=pq,
                    func=AF.Identity,
                    bias=bq_sb[:, t : t + 1],
                    scale=1.0 / WSCALE,
                )

            KTt = [
                ktp.tile([128, SK], BF16, tag="kt", name=f"KT_{t}")
                for t in range(8)
            ]

            def emit_kproj(t, jc, fromfill=True):
                pk = proj_psum(f"pk_{t}_{jc}", fromfill)
                for p in range(4):
                    nc.tensor.matmul(
                        pk,
                        wkv[:, 2 * p : 2 * p + 2, t * 128 : (t + 1) * 128],
                        kv[:, 2 * p : 2 * p + 2, jc * 512 : (jc + 1) * 512],
                        start=(p == 0),
                        stop=(p == 3),
                        perf_mode=DR,
                    )
                nc.scalar.activation(
                    out=KTt[t][:, jc * 512 : (jc + 1) * 512],
                    in_=pk,
                    func=AF.Identity,
                    bias=bk_sb[:, t : t + 1],
                    scale=1.0 / WSCALE,
                )

            # V8 tiles: per jt-pair [128 j, 2 slot, 16 head, 65] fp8e4
            v8t = [
                vsb.tile([128, 2, H, 65], F8E4, tag="v8", name=f"V8_{pr}")
                for pr in range(8)
            ]
            for pr in range(8):
                nc.vector.memset(v8t[pr][:, :, :, 64:65], 1.0)

            def emit_vproj(dh, jb, fromfill=True):
                pv = proj_psum(f"pv_{dh}_{jb}", fromfill)
                for p in range(4):
                    nc.tensor.matmul(
                        pv,
                        vv[:, jb, 2 * p : 2 * p + 2, :],
                        wvv[:, 2 * p : 2 * p + 2, dh * 512 : (dh + 1) * 512],
                        start=(p == 0),
                        stop=(p == 3),
                        perf_mode=DR,
                    )
                v8dst = v8t[jb // 2][:, jb % 2, dh * 8 : (dh + 1) * 8, 0:64]
                pvv = pv.rearrange("p (h c) -> p h c", h=8)
                if jb % 2 == 0:
                    nc.scalar.activation(
                        out=v8dst, in_=pvv, func=AF.Copy, scale=1.0 / WSCALE
                    )
                else:
                    nc.vector.tensor_scalar(
                        out=v8dst,
                        in0=pvv,
                        scalar1=1.0 / WSCALE,
                        scalar2=None,
                        op0=OP.mult,
                    )

            # late-needed constants
            wo_t = inp.tile([128, 8 * E], F8E4, tag="wo")
            nc.sync.dma_start(out=wo_t, in_=wo8[:, :])
            wov = wo_t.rearrange("p (s c) -> p s c", s=8)
            res_t = [
                inp.tile([128, E], BF16, tag="res", name=f"res_{qb}", bufs=4)
                for qb in range(4)
            ]
            for qb in range(4):
                nc.sync.dma_start(
                    out=res_t[qb], in_=resid[qb * 128 : (qb + 1) * 128, :]
                )
            gamma_b = constp.tile([128, E], BF16)
            nc.sync.dma_start(out=gamma_b, in_=vec3[0, :].partition_broadcast(128))
            beta_b = constp.tile([128, E], BF16)
            nc.sync.dma_start(out=beta_b, in_=vec3[1, :].partition_broadcast(128))
            id_t = constp.tile([128, 128], BF16)
            nc.sync.dma_start(out=id_t, in_=ident[:, :])

            # ctx8: [128 d, 8 d-tile slot, 512 q] fp8e4 (x8 scale)
            ctx8 = ctxp.tile([128, 8, QR], F8E4, tag="ctx8")

            # ---------------- attention heads ----------------
            def emit_head(h, fill=None):
                kt = KTt[h // 2]
                qt = QTt[h // 2]
                r0 = 64 * (h % 2)
                eng = ENG_EARLY if h < 8 else ENG_LATE
                pc = pcp.tile([128, 512], F32, tag="pc", name=f"pc_{h}")
                for pair in range(8):
                    if fill is not None:
                        fill(pair)
                    et = expp.tile([128, 2, 512], F8E5, tag="exp", name=f"e_{h}_{pair}")
                    sp = psc.tile([128, 2, 512], F32, tag="sc", name=f"s_{h}_{pair}")
                    for s in range(2):
                        jt = 2 * pair + s
                        nc.tensor.matmul(
                            sp[:, s, :],
                            kt[r0 : r0 + 64, jt * 128 : (jt + 1) * 128],
                            qt[r0 : r0 + 64, :],
                            start=True,
                            stop=True,
                        )
                    if eng[pair] == "A":
                        nc.scalar.activation(
                            out=et[:], in_=sp[:], func=AF.Exp, scale=0.125
                        )
                    else:
                        nc.vector.tensor_scalar(
                            out=et[:].bitcast(I8),
                            in0=sp[:],
                            scalar1=EXPA,
                            scalar2=EXPB,
                            op0=OP.mult,
                            op1=OP.add,
                        )
                    nc.tensor.matmul(
                        pc[0:65, :],
                        v8t[pair][:, :, h, :],
                        et[:],
                        start=(pair == 0),
                        stop=(pair == 7),
                        perf_mode=DR,
                    )
                # normalize: ctx8[d, i] = CTXSCALE * ctx~[d, i] / denom[i]
                rec = recp.tile([1, 512], F32, tag="rec", name=f"rc_{h}")
                nc.vector.reciprocal(out=rec, in_=pc[64:65, :])
                rb = recp.tile([64, 512], F32, tag="rb", name=f"rb_{h}")
                nc.gpsimd.partition_broadcast(rb[:], rec[:], channels=64)
                nc.vector.scalar_tensor_tensor(
                    out=ctx8[r0 : r0 + 64, h // 2, :],
                    in0=pc[0:64, :],
                    scalar=CTXSCALE,
                    in1=rb[:],
                    op0=OP.mult,
                    op1=OP.mult,
                )

            # fill queue: remaining K tiles (1..7) and V d-half 1, emitted
            # inside the ACT-bound head loops, paced by deadline so early
            # heads aren't congested. K tile t must land before head 2t
            # (slot 16t); V dh1 before head 8 (slot 64).
            # pace fills so none is emitted while its input DMA is still in
            # flight (a DMA-gated fill tile blocks the whole scores-pool
            # rotation): nothing before slot 8 (head 1).
            fills = []
            for t in range(1, 8):
                for jc in range(4):
                    fills.append((max(8, 16 * (t - 1)) + 2 * jc, "K", t, jc))
            for jb in range(16):
                fills.append((16 + 3 * jb, "V", 1, jb))
            fills.sort(key=lambda it: it[0])
            fill_state = {"i": 0, "slot": 0}

            def fill_one(_pair):
                s = fill_state["slot"]
                fill_state["slot"] = s + 1
                while fill_state["i"] < len(fills) and fills[fill_state["i"]][0] <= s:
                    _, kind, a, b2 = fills[fill_state["i"]]
                    fill_state["i"] += 1
                    if kind == "K":
                        emit_kproj(a, b2)
                    else:
                        emit_vproj(a, b2)

            # prologue: K tile 0 + V d-half 0 (through the pc pool so the
            # scores-pool rotation isn't blocked behind DMA-gated projections)
            for jc in range(4):
                emit_kproj(0, jc, fromfill=False)
            for jb in range(16):
                emit_vproj(0, jb, fromfill=False)

            for h in range(16):
                emit_head(h, fill=fill_one if h < 14 else None)
            # drain any leftover fills
            while fill_state["i"] < len(fills):
                _, kind, a, b2 = fills[fill_state["i"]]
                fill_state["i"] += 1
                if kind == "K":
                    emit_kproj(a, b2)
                else:
                    emit_vproj(a, b2)

            # ------------- out projection + residual + LayerNorm -------------
            # The residual is folded into the out-proj PSUM group by a final
            # 256*I matmul, so PSUM holds 256*(out+resid) = 256*x; LN is
            # scale-invariant once eps is scaled by 256^2. rstd comes from a
            # DVE rsqrt bit-trick + one Newton step (no ACT tables), and the
            # (x-mu)*rstd affine runs on ACT via Identity(scale, bias).
            MAGIC = float(0x5F3759DF)
            EPS256 = EPS * 65536.0
            for qb in range(4):
                pos = []
                st = lnp.tile([128, 2, 6], F32, tag="st", name=f"st_{qb}")
                po2 = psc.tile([128, 2, 512], F32, tag="sc", name=f"po_{qb}")
                for eh in range(2):
                    po = po2[:, eh, :]
                    for p in range(4):
                        nc.tensor.matmul(
                            po[:],
                            ctx8[:, 2 * p : 2 * p + 2, qb * 128 : (qb + 1) * 128],
                            wov[:, 2 * p : 2 * p + 2, eh * 512 : (eh + 1) * 512],
                            start=(p == 0),
                            stop=False,
                            perf_mode=DR,
                        )
                    nc.tensor.matmul(
                        po,
                        id_t[:],
                        res_t[qb][:, eh * 512 : (eh + 1) * 512],
                        start=False,
                        stop=True,
                    )
                    nc.vector.bn_stats(out=st[:, eh, :], in_=po)
                    pos.append(po)
                mv = lnp.tile([128, 2], F32, tag="mv", name=f"mv_{qb}", bufs=4)
                nc.vector.bn_aggr(out=mv, in_=st)
                # rstd = rsqrt(var + eps) via bit trick + one Newton step
                w = lnp.tile([128, 4], F32, tag="w", name=f"w_{qb}", bufs=4)
                nc.vector.tensor_scalar(
                    out=w[:, 0:1], in0=mv[:, 1:2], scalar1=EPS256,
                    scalar2=None, op0=OP.add,
                )
                nc.vector.tensor_scalar(
                    out=w[:, 1:2].bitcast(mybir.dt.int32),
                    in0=w[:, 0:1].bitcast(mybir.dt.int32),
                    scalar1=-0.5, scalar2=MAGIC, op0=OP.mult, op1=OP.add,
                )
                nc.vector.tensor_tensor(
                    out=w[:, 2:3], in0=w[:, 1:2], in1=w[:, 1:2], op=OP.mult
                )
                nc.vector.tensor_tensor(
                    out=w[:, 2:3], in0=w[:, 2:3], in1=w[:, 0:1], op=OP.mult
                )
                nc.vector.tensor_scalar(
                    out=w[:, 2:3], in0=w[:, 2:3],
                    scalar1=-0.5, scalar2=1.5, op0=OP.mult, op1=OP.add,
                )
                nc.vector.tensor_tensor(
                    out=w[:, 2:3], in0=w[:, 1:2], in1=w[:, 2:3], op=OP.mult
                )
                # w2 = rstd(256x); nmr = -mean(256x)*rstd
                nc.vector.tensor_scalar(
                    out=w[:, 3:4], in0=mv[:, 0:1],
                    scalar1=w[:, 2:3], scalar2=-1.0, op0=OP.mult, op1=OP.mult,
                )
                a = lnp.tile([128, E], BF16, tag="a", name=f"a_{qb}")
                for eh in range(2):
                    nc.scalar.activation(
                        out=a[:, eh * 512 : (eh + 1) * 512],
                        in_=pos[eh],
                        func=AF.Identity,
                        bias=w[:, 3:4],
                        scale=w[:, 2:3],
                    )
                y = lnp.tile([128, E], BF16, tag="y", name=f"y_{qb}")
                nc.gpsimd.tensor_tensor(out=y, in0=a, in1=gamma_b, op=OP.mult)
                nc.vector.tensor_tensor(out=y, in0=y, in1=beta_b, op=OP.add)
                nc.sync.dma_start(
                    out=out[qb * 128 : (qb + 1) * 128, :], in_=y
                )

    nc.finalize()
    return nc


def _interleave_etiles(arr):
    """[E, N] -> [128, 8*N] with e = slot*128 + partition pairing layout."""
    Edim, N = arr.shape
    return np.ascontiguousarray(
        arr.reshape(8, 128, N).transpose(1, 0, 2).reshape(128, 8 * N)
    )


def build_in_maps(inputs):
    q = np.asarray(inputs["query"], dtype=np.float32)
    k = np.asarray(inputs["key"], dtype=np.float32)
    v = np.asarray(inputs["value"], dtype=np.float32)
    Wq = np.asarray(inputs["Wq"], dtype=np.float32)
    bq = np.asarray(inputs["bq"], dtype=np.float32)
    Wk = np.asarray(inputs["Wk"], dtype=np.float32)
    bk = np.asarray(inputs["bk"], dtype=np.float32)
    Wv = np.asarray(inputs["Wv"], dtype=np.float32)
    bv = np.asarray(inputs["bv"], dtype=np.float32)
    Wo = np.asarray(inputs["Wo"], dtype=np.float32)
    bo = np.asarray(inputs["bo"], dtype=np.float32)
    gamma = np.asarray(inputs["gamma"], dtype=np.float32)
    beta = np.asarray(inputs["beta"], dtype=np.float32)

    e4 = ml_dtypes.float8_e4m3
    # weights: pre-scaled x32, e = slot*128 + partition layout
    wq8 = _interleave_etiles(Wq.T * WSCALE).astype(e4)
    wk8 = _interleave_etiles(Wk.T * WSCALE).astype(e4)
    wv8 = _interleave_etiles(Wv.T * WSCALE).astype(e4)
    wo8 = _interleave_etiles(Wo.T * WSCALE).astype(e4)

    # per-batch activations
    k8 = [_interleave_etiles(np.ascontiguousarray(k[b].T)).astype(e4) for b in range(B)]
    v8 = []
    for b in range(B):
        t = _interleave_etiles(np.ascontiguousarray(v[b].T))  # [128, 8*2048]
        t = (
            t.reshape(128, 8, 16, 128)
            .transpose(0, 2, 1, 3)
            .reshape(128, 16 * 8 * 128)
        )
        v8.append(np.ascontiguousarray(t).astype(e4))

    # bv folded into a host-side bias vector: out includes +bv @ Wo.T + bo.
    bo_eff = (bv @ Wo.T + bo).astype(np.float32)

    in_maps = []
    for c in range(N_CORES):
        b, g = divmod(c, 2)
        rows = slice(QR * g, QR * g + QR)
        q8 = _interleave_etiles(np.ascontiguousarray(q[b, rows, :].T)).astype(e4)
        in_maps.append(
            {
                "q8": q8,
                "k8": k8[b],
                "v8": v8[b],
                "wq8": wq8,
                "wk8": wk8,
                "wv8": wv8,
                "wo8": wo8,
                "bq8": np.ascontiguousarray(bq.reshape(8, 128).T),
                "bk8": np.ascontiguousarray(bk.reshape(8, 128).T),
                "resid": np.ascontiguousarray(q[b, rows, :] + bo_eff).astype(
                    ml_dtypes.bfloat16
                ),
                "ident": (np.eye(128, dtype=np.float32) * (WSCALE * CTXSCALE)).astype(
                    ml_dtypes.bfloat16
                ),
                "vec3": np.ascontiguousarray(np.stack([gamma, beta])).astype(
                    ml_dtypes.bfloat16
                ),
            }
        )
    return in_maps


def kernel(**inputs):
    global _NC_CACHE, LAST_RESULTS
    if _NC_CACHE is None:
        _NC_CACHE = _build_nc()
    nc = _NC_CACHE

    in_maps = build_in_maps(inputs)

    res = run_bass_kernel_spmd(nc, in_maps, list(range(N_CORES)), trace=TRACE)
    LAST_RESULTS = res

    outp = np.empty((B, SQ, E), dtype=np.float32)
    for c in range(N_CORES):
        b, g = divmod(c, 2)
        outp[b, QR * g : QR * g + QR, :] = res.results[c]["out"].astype(np.float32)
    return outp
